# revision 4
# baseline (speedup 1.0000x reference)
"""Bass/Trainium2 kernel for nn_GreedyMatcher: batched PDHG LP solver.

Reference computation (per batch sample b):
    B = X.reshape(bs, 128); Wb = broadcast(W)
    x0 = y0 = 0, xbar0 = 0
    repeat 100x:
        y   = relu(y + sigma*(xbar @ S.T - B))
        x'  = relu(x + tau*(W - y @ S))
        xbar = 2x' - x ; x = x'
    return x  [bs, 2048]

Strategy: pure data parallel over batch (256 -> 32 per core, 8 cores).
Per-core state is kept struct-major in SBUF: Xsb[p, 32*m + b] = x[b, 128*m + p]
so both matmuls per iteration run with K=128 chunks on the tensor engine.
The extrapolation xbar is never materialized: with V_t = S @ x_t^T,
S @ xbar_t^T = 2 V_t - V_{t-1}, and the dual update folds into a single
carried tensor g_t = y_t - sigma*V_{t-1} - sigma*B^T:
    v      = (2 sigma V_t) + g_t          # psum + g
    y_.    = relu(v)
    g_{t+1}= y_. - sigma*V_t - sigma*B^T
    x_{t+1}= relu(x_t + tau - tau*(S^T y_.))   (W == ones fast path)
"""

import sys
import os

sys.path.insert(0, "/opt/trn_rl_repo")

import numpy as np

N_CORES = 8
BATCH = 256
BS = BATCH // N_CORES  # 32 per core
N_HOS = 8
N_TYPES = 16
M_CONS = N_HOS * N_TYPES  # 128 constraints
N_STRUCTS = 2048
N_CHUNKS = N_STRUCTS // 128  # 16
N_ITERS = 100

_CACHE = {}


def _spec_norm_f32(S: np.ndarray) -> np.float32:
    """Mimic reference._spec_norm in float32 numpy."""
    S = S.astype(np.float32)
    v = np.ones((S.shape[1],), np.float32)
    v = v / np.float32(np.linalg.norm(v))
    for _ in range(30):
        u = S @ v
        u = u / (np.float32(np.linalg.norm(u)) + np.float32(1e-12))
        v = S.T @ u
        v = v / (np.float32(np.linalg.norm(v)) + np.float32(1e-12))
    return np.float32(np.linalg.norm(S @ v))


def _get_compiled(tau: float, mm_dtype_name: str, w_is_ones: bool):
    key = (round(float(tau), 10), mm_dtype_name, w_is_ones)
    if key in _CACHE:
        return _CACHE[key]
    nc = _build_real(mm_dtype_name, w_is_ones, float(tau))
    nc.compile()
    _CACHE[key] = nc
    return nc


def _build_real(mm_dtype_name: str, w_is_ones: bool, tau: float):
    import concourse.bacc as bacc
    import concourse.tile as tile
    import concourse.mybir as mybir
    from contextlib import ExitStack

    f32 = mybir.dt.float32
    mmdt = getattr(mybir.dt, mm_dtype_name)
    ALU = mybir.AluOpType

    nc = bacc.Bacc(None, target_bir_lowering=False)

    STs_d = nc.dram_tensor("STs", [128, N_STRUCTS], mmdt, kind="ExternalInput")
    Ss_d = nc.dram_tensor("Ss", [128, N_STRUCTS], mmdt, kind="ExternalInput")
    Bs_d = nc.dram_tensor("Bs", [128, BS], f32, kind="ExternalInput")
    if not w_is_ones:
        TW_d = nc.dram_tensor("TW", [128, N_CHUNKS * BS], f32, kind="ExternalInput")
    XO_d = nc.dram_tensor("XOUT", [128, N_CHUNKS * BS], f32, kind="ExternalOutput")

    FD = N_CHUNKS * BS

    with tile.TileContext(nc) as tc:
        with ExitStack() as ctx:
            const = ctx.enter_context(tc.tile_pool(name="const", bufs=1))
            state = ctx.enter_context(tc.tile_pool(name="state", bufs=1))
            tmp = ctx.enter_context(tc.tile_pool(name="tmp", bufs=3))
            psum = ctx.enter_context(tc.tile_pool(name="psum", bufs=2, space="PSUM"))

            STs = const.tile([128, N_STRUCTS], mmdt, tag="STs")
            Ss = const.tile([128, N_STRUCTS], mmdt, tag="Ss")
            Bs = const.tile([128, BS], f32, tag="Bs")
            nc.sync.dma_start(STs[:], STs_d[:])
            nc.sync.dma_start(Ss[:], Ss_d[:])
            nc.sync.dma_start(Bs[:], Bs_d[:])
            if not w_is_ones:
                TW = const.tile([128, FD], f32, tag="TW")
                nc.sync.dma_start(TW[:], TW_d[:])

            xb = [state.tile([128, FD], mmdt, name=f"x{i}", tag=f"x{i}") for i in range(2)]
            gb = [state.tile([128, BS], f32, name=f"g{i}", tag=f"g{i}") for i in range(2)]

            nc.gpsimd.memset(xb[0][:], 0.0)
            nc.vector.tensor_scalar_mul(gb[0][:], Bs[:], -1.0)

            for t in range(N_ITERS):
                x_cur = xb[t % 2]
                x_nxt = xb[(t + 1) % 2]
                g_cur = gb[t % 2]
                g_nxt = gb[(t + 1) % 2]

                pV = psum.tile([128, BS], f32, tag="pV")
                for k in range(N_CHUNKS):
                    nc.tensor.matmul(
                        pV[:],
                        STs[:, 128 * k : 128 * (k + 1)],
                        x_cur[:, BS * k : BS * (k + 1)],
                        start=(k == 0),
                        stop=(k == N_CHUNKS - 1),
                    )
                v = tmp.tile([128, BS], f32, tag="v")
                nc.vector.tensor_add(v[:], pV[:], g_cur[:])
                ynew = tmp.tile([128, BS], mmdt, tag="y")
                nc.vector.tensor_scalar_max(ynew[:], v[:], 0.0)

                yB = tmp.tile([128, BS], f32, tag="yB")
                nc.vector.tensor_sub(yB[:], ynew[:], Bs[:])
                nc.vector.scalar_tensor_tensor(
                    g_nxt[:], pV[:], -0.5, yB[:], ALU.mult, ALU.add
                )

                pX = psum.tile([128, FD], f32, tag="pX")
                for m in range(N_CHUNKS):
                    nc.tensor.matmul(
                        pX[:, BS * m : BS * (m + 1)],
                        Ss[:, 128 * m : 128 * (m + 1)],
                        ynew[:],
                        start=True,
                        stop=True,
                    )
                e = tmp.tile([128, FD], f32, tag="e")
                if w_is_ones:
                    nc.vector.scalar_tensor_tensor(
                        e[:], x_cur[:], tau, pX[:], ALU.add, ALU.subtract
                    )
                else:
                    nc.vector.tensor_sub(e[:], TW[:], pX[:])
                    nc.vector.tensor_add(e[:], e[:], x_cur[:])
                nc.vector.tensor_scalar_max(x_nxt[:], e[:], 0.0)

            x_fin = xb[N_ITERS % 2]
            if mm_dtype_name == "float32":
                nc.sync.dma_start(XO_d[:], x_fin[:])
            else:
                xf = tmp.tile([128, FD], f32, tag="xf")
                nc.vector.tensor_copy(xf[:], x_fin[:])
                nc.sync.dma_start(XO_d[:], xf[:])

    return nc


MM_DTYPE = os.environ.get("GM_MM_DTYPE", "float32")


def kernel_run(X, S, W, batch_size, trace=False, tmpdir=None):
    from concourse.bass_utils import run_bass_kernel_spmd

    X = np.asarray(X, np.float32)
    S = np.asarray(S, np.float32)
    W = np.asarray(W, np.float32)
    bs = int(batch_size)
    assert bs == BATCH and X.shape == (BATCH, N_HOS, N_TYPES)
    assert S.shape == (M_CONS, N_STRUCTS)

    L = _spec_norm_f32(S)
    sigma = np.float32(0.9) / L
    tau = np.float32(0.9) / L

    B = X.reshape(BATCH, M_CONS)
    w_is_ones = bool(np.all(W == 1.0))

    np_mmdt = {"float32": np.float32, "float16": np.float16}[MM_DTYPE]
    # STs[p, 128k+j] = 2*sigma*S[j, 128k+p]
    STs = (
        (np.float32(2.0) * sigma * S)
        .T.reshape(N_CHUNKS, 128, 128)
        .transpose(1, 0, 2)
        .reshape(128, N_STRUCTS)
        .astype(np_mmdt)
    )
    Ss = (tau * S).astype(np_mmdt)

    in_maps = []
    for c in range(N_CORES):
        Bs_c = (sigma * B[BS * c : BS * (c + 1), :]).T.astype(np.float32)
        m = {"STs": STs, "Ss": Ss, "Bs": np.ascontiguousarray(Bs_c)}
        if not w_is_ones:
            TW_c = np.broadcast_to(
                (tau * W).reshape(N_CHUNKS, 128, 1), (N_CHUNKS, 128, BS)
            )
            m["TW"] = np.ascontiguousarray(
                TW_c.transpose(1, 0, 2).reshape(128, N_CHUNKS * BS).astype(np.float32)
            )
        in_maps.append(m)

    nc = _get_compiled(float(tau), MM_DTYPE, w_is_ones)
    res = run_bass_kernel_spmd(
        nc, in_maps, list(range(N_CORES)), trace=trace, tmpdir=tmpdir
    )

    out = np.empty((BATCH, N_STRUCTS), np.float32)
    for c in range(N_CORES):
        O = res.results[c]["XOUT"]  # [128, 512]
        out[BS * c : BS * (c + 1), :] = (
            O.reshape(128, N_CHUNKS, BS).transpose(2, 1, 0).reshape(BS, N_STRUCTS)
        )
    return out, res


def kernel(**inputs):
    out, _ = kernel_run(
        inputs["X"], inputs["S"], inputs["W"], inputs["batch_size"], trace=False
    )
    return out
